# revision 5
# baseline (speedup 1.0000x reference)
"""Trainium2 Bass kernel for the scatter_memory problem.

Full (unsharded) inputs in, full output out. Internally: 8-way shard over
(batch, window-half); pair-wise AllReduce combines softmax partials.

Math restructuring vs the reference (validated to rel err 5e-6 in fp32):
  - the self-attention branch (sa_*) is dead code -> skipped
  - summary feeds only the cross-attention; scores fold qa_q/sqrt(d) @ qa_wk
    into one [1024, 64] matrix on the host
  - softmax without max-subtraction (scores are in [-6, 6]); partial
    numerator/denominator sums are combined with a pair AllReduce
"""

import numpy as np

import concourse.bacc as bacc
import concourse.mybir as mybir
import concourse.tile as tile
import concourse.bass_utils as bass_utils

N_CORES = 8
DIM = 1024
L = 16            # SUMMARY_LEN
STRIDE = 8
NWIN = 512        # windows per batch
NLOC = 256        # windows per core (half a batch)
XLOC = NLOC * STRIDE + (L - STRIDE)   # 2056 x-positions per core
CONV = 4104       # padded seq len
EPS = 1.1920929e-07
BF16 = True       # dtype of the big-projection operands (x windows, ws_w)

_DT = mybir.dt.bfloat16 if BF16 else mybir.dt.float32
_NPDT = np.dtype("bfloat16") if BF16 else np.dtype("float32")


def build_nc(reps: int = 1, use_collective: bool = True):
    """Build the per-core Bass module. `reps` statically repeats the whole
    body (for wall-clock-delta timing). With use_collective=False the pair
    combine becomes a local copy (for single-core simulation)."""
    f32 = mybir.dt.float32
    nc = bacc.Bacc("TRN2", target_bir_lowering=False, debug=False,
                   num_devices=N_CORES)

    xt_d = nc.dram_tensor("xt", [8, 128, XLOC], _DT, kind="ExternalInput")
    wt_d = nc.dram_tensor("wt", [128, 128, 1024], _DT, kind="ExternalInput")
    wv_d = nc.dram_tensor("wv", [8, 128, 1024], f32, kind="ExternalInput")
    cq_d = nc.dram_tensor("cq", [8, 128, 64], f32, kind="ExternalInput")
    h_d = nc.dram_tensor("hb", [64, 1024], f32, kind="ExternalInput")
    mn_d = nc.dram_tensor("mnw", [64, 1024], f32, kind="ExternalInput")
    hn_d = nc.dram_tensor("hnw", [64, 1024], f32, kind="ExternalInput")
    id_d = nc.dram_tensor("ident", [64, 64], f32, kind="ExternalInput")
    out_d = nc.dram_tensor("out", [64, 1024], f32, kind="ExternalOutput")

    with tile.TileContext(nc) as tc:
        with (
            tc.tile_pool(name="const", bufs=1) as cpool,
            tc.tile_pool(name="x", bufs=1) as xpool,
            tc.tile_pool(name="w", bufs=4) as wpool,
            tc.tile_pool(name="sm", bufs=1) as spool,
            tc.tile_pool(name="small", bufs=2) as mpool,
            tc.tile_pool(name="ps", bufs=8, space="PSUM") as ppool,
            tc.tile_pool(name="dram", bufs=2, space="DRAM") as dpool,
        ):
            ident = cpool.tile([64, 64], f32, tag="ident")
            nc.sync.dma_start(ident[:], id_d[:])
            ones = cpool.tile([128, 64], f32, tag="ones")
            nc.vector.memset(ones[:], 1.0)
            eps_sb = cpool.tile([1, 1], f32, tag="eps")
            nc.vector.memset(eps_sb[:], EPS)

            for _rep in range(reps):
                # ---- load per-core inputs ----
                xts = xpool.tile([128, 8, XLOC], _DT, tag="xts")
                for dc in range(8):
                    nc.sync.dma_start(xts[:, dc, :], xt_d[dc])
                wv_sb = cpool.tile([128, 8, 1024], f32, tag="wv")
                nc.sync.dma_start(wv_sb[:], wv_d[:].rearrange("a b c -> b a c"))
                cq_sb = cpool.tile([128, 8, 64], f32, tag="cq")
                nc.sync.dma_start(cq_sb[:], cq_d[:].rearrange("a b c -> b a c"))
                h_sb = mpool.tile([64, 1024], f32, tag="h")
                nc.sync.dma_start(h_sb[:], h_d[:])
                mn_sb = mpool.tile([64, 1024], f32, tag="mn")
                nc.sync.dma_start(mn_sb[:], mn_d[:])
                hn_sb = mpool.tile([64, 1024], f32, tag="hn")
                nc.sync.dma_start(hn_sb[:], hn_d[:])

                # ---- projection: summaryT[m, n] = sum_f wsT[f, m] win[n, f] ----
                ps = [ppool.tile([128, 256], f32, tag="ps", name=f"ps{i}") for i in range(8)]
                for f in range(128):
                    l, dc = f // 8, f % 8
                    wch = wpool.tile([128, 1024], _DT, tag="wch")
                    nc.sync.dma_start(wch[:], wt_d[f])
                    rhs = xts[:, dc, l:l + 2041:8]   # [128, 256] stride-8 view
                    for mt in range(8):
                        nc.tensor.matmul(
                            ps[mt][:], wch[:, mt * 128:(mt + 1) * 128], rhs,
                            start=(f == 0), stop=(f == 127))
                sm = [spool.tile([128, 256], f32, tag=f"sm{mt}", name=f"sm{mt}") for mt in range(8)]
                for mt in range(8):
                    nc.vector.tensor_copy(sm[mt][:], ps[mt][:])

                # ---- qv[n, h] = summary @ qa_wv^T ----
                qv_sb = [spool.tile([128, 1024], f32, tag=f"qv{nt}", name=f"qv{nt}") for nt in range(2)]
                for nt in range(2):
                    for hh in range(2):
                        qp = ppool.tile([128, 512], f32, tag="ps")
                        for mt in range(8):
                            nc.tensor.matmul(
                                qp[:], sm[mt][:, nt * 128:(nt + 1) * 128],
                                wv_sb[:, mt, hh * 512:(hh + 1) * 512],
                                start=(mt == 0), stop=(mt == 7))
                        nc.vector.tensor_copy(qv_sb[nt][:, hh * 512:(hh + 1) * 512], qp[:])

                # ---- scores[q, n] (pre-scaled), exp, row sums ----
                sc_ps = ppool.tile([64, 256], f32, tag="ps")
                for mt in range(8):
                    nc.tensor.matmul(sc_ps[:], cq_sb[:, mt, :], sm[mt][:],
                                     start=(mt == 0), stop=(mt == 7))
                p_sb = mpool.tile([64, 256], f32, tag="p")
                sloc = mpool.tile([64, 1], f32, tag="sloc")
                nc.scalar.activation(p_sb[:], sc_ps[:],
                                     mybir.ActivationFunctionType.Exp,
                                     accum_out=sloc[:])

                # ---- P^T via PE transpose ----
                pt_sb = []
                for nt in range(2):
                    tp = ppool.tile([128, 64], f32, tag="ps")
                    nc.tensor.transpose(tp[:], p_sb[:, nt * 128:(nt + 1) * 128], ident[:])
                    t_sb = mpool.tile([128, 64], f32, tag=f"pt{nt}")
                    nc.vector.tensor_copy(t_sb[:], tp[:])
                    pt_sb.append(t_sb)

                # ---- out_loc[q, h] = P^T.T @ qv (unnormalized) ----
                payload = mpool.tile([64, 1025], f32, tag="payload")
                for hh in range(2):
                    op = ppool.tile([64, 512], f32, tag="ps")
                    for nt in range(2):
                        nc.tensor.matmul(op[:], pt_sb[nt][:],
                                         qv_sb[nt][:, hh * 512:(hh + 1) * 512],
                                         start=(nt == 0), stop=(nt == 1))
                    nc.vector.tensor_copy(payload[:, hh * 512:(hh + 1) * 512], op[:])
                nc.vector.tensor_copy(payload[:, 1024:1025], sloc[:])

                # ---- pair AllReduce of (numerator, denominator) ----
                comb = mpool.tile([64, 1025], f32, tag="comb")
                if use_collective:
                    cin = dpool.tile([64, 1025], f32, tag="cin")
                    cout = dpool.tile([64, 1025], f32, tag="cout")
                    nc.sync.dma_start(cin[:], payload[:])
                    nc.gpsimd.collective_compute(
                        "AllReduce", mybir.AluOpType.add,
                        replica_groups=[[0, 1], [2, 3], [4, 5], [6, 7]],
                        ins=[cin.opt()], outs=[cout.opt()])
                    nc.sync.dma_start(comb[:], cout[:])
                else:
                    nc.vector.tensor_copy(comb[:], payload[:])

                # ---- memory = num / den; two full-tensor RMSNorms ----
                rec = mpool.tile([64, 1], f32, tag="rec")
                nc.vector.reciprocal(rec[:], comb[:, 1024:1025])
                mem = mpool.tile([64, 1024], f32, tag="mem")
                nc.vector.tensor_scalar_mul(mem[:], comb[:, 0:1024], rec[:])

                def rms2d(t_in, w_sb, add_sb=None):
                    # t_in * rsqrt(mean(t_in^2) + EPS) * w  (mean over all 64*1024)
                    scratch = mpool.tile([64, 1024], f32, tag="scratch")
                    sq = mpool.tile([64, 1], f32, tag="sq")
                    nc.scalar.activation(scratch[:], t_in[:],
                                         mybir.ActivationFunctionType.Square,
                                         accum_out=sq[:])
                    msp = ppool.tile([1, 1], f32, tag="ps")
                    nc.tensor.matmul(msp[:], sq[:], ones[0:64, 0:1],
                                     start=True, stop=True)
                    std = mpool.tile([1, 1], f32, tag="std")
                    nc.scalar.activation(std[:], msp[:],
                                         mybir.ActivationFunctionType.Sqrt,
                                         scale=1.0 / 65536.0, bias=eps_sb[:])
                    bst = ppool.tile([64, 1], f32, tag="ps")
                    nc.tensor.matmul(bst[:], ones[0:1, 0:64], std[:],
                                     start=True, stop=True)
                    rstd = mpool.tile([64, 1], f32, tag="rstd")
                    nc.vector.reciprocal(rstd[:], bst[:])
                    o = mpool.tile([64, 1024], f32, tag="rmsout")
                    nc.vector.tensor_scalar_mul(o[:], t_in[:], rstd[:])
                    nc.vector.tensor_mul(o[:], o[:], w_sb[:])
                    if add_sb is not None:
                        nc.vector.tensor_add(o[:], o[:], add_sb[:])
                    return o

                hh1 = rms2d(mem, mn_sb, add_sb=h_sb)
                o = rms2d(hh1, hn_sb)
                nc.sync.dma_start(out_d[:], o[:])

    nc.compile()
    return nc


def prep_inputs(x, h, ws_w, qa_q, qa_wk, qa_wv, mn_w, hn_w):
    """Host-side slicing/transposes -> per-core input maps."""
    bsz = x.shape[0]
    xp = np.zeros((bsz, CONV, DIM), np.float32)
    xp[:, :x.shape[1], :] = x
    wt = np.ascontiguousarray(ws_w.T).reshape(128, 128, 1024).astype(_NPDT)
    wv = np.ascontiguousarray(qa_wv.T).reshape(8, 128, 1024)
    cq = np.ascontiguousarray(
        ((qa_q.astype(np.float64) / np.sqrt(np.float64(DIM))).astype(np.float32)
         @ qa_wk).T).reshape(8, 128, 64)
    ident = np.eye(64, dtype=np.float32)
    in_maps = []
    for c in range(N_CORES):
        b, half = c // 2, c % 2
        p0 = half * NLOC * STRIDE
        xt = np.ascontiguousarray(
            xp[b, p0:p0 + XLOC, :].T).reshape(8, 128, XLOC).astype(_NPDT)
        in_maps.append({
            "xt": xt, "wt": wt, "wv": wv, "cq": cq,
            "hb": np.ascontiguousarray(h[b]),
            "mnw": np.ascontiguousarray(mn_w),
            "hnw": np.ascontiguousarray(hn_w),
            "ident": ident,
        })
    return in_maps


_NC_CACHE = {}


def kernel(x, h, ws_w, sa_wq, sa_wk, sa_wv, qa_q, qa_wk, qa_wv, mn_w, hn_w):
    if "nc" not in _NC_CACHE:
        _NC_CACHE["nc"] = build_nc(reps=1, use_collective=True)
    nc = _NC_CACHE["nc"]
    in_maps = prep_inputs(x, h, ws_w, qa_q, qa_wk, qa_wv, mn_w, hn_w)
    res = bass_utils.run_bass_kernel_spmd(nc, in_maps, core_ids=list(range(N_CORES)))
    out = np.stack([res.results[2 * b]["out"] for b in range(4)], axis=0)
    return out.astype(np.float32)
